# revision 10
# baseline (speedup 1.0000x reference)
"""Distributed Trainium2 Bass kernel for the contextual-attention module.

Strategy (per sharding hint): data-parallel over batch (2 samples x 4 cores),
within a sample the L=4096 patch/kernel axis is sharded 4 ways (1024 kernels
per core, = 16 rows of patch centers).  Per core:

  scores[l, s]  = sum_{c,dy,dx} kern_bf[l,c,dy,dx] * boxfeat_bf[c, y+dy, x+dx]
  (the reference's 3x3 box-sum of scores is commuted into a 3x3 box filter
   of the feature map, so it rides along in the same GEMM)
  kernel L2 normalization is folded in as a per-l row scale (rnorm) applied
  to scores (pre-softmax) and to attn (pre-transpose-conv).
  softmax over the full L axis = local max/sum + AllReduce(max) +
  AllReduce(add) of per-spatial-position stats over the 4-core group.
  transpose-conv: per (dy,dx), partial[c, s] = kern^T @ attn accumulated in
  PSUM, overlap-added into a padded canvas; final blend
  out = canvas*(1-mask)/9 + feat*mask/4 (the /4 makes the feat term sum to
  1x across the group) followed by a 4-core ReduceScatter over channels.

Each core returns a [32, 4096] channel band; the host stitches the full
[2, 128, 64, 64] output.
"""

import os
import sys
import types

for _p in ("/opt/trn_rl_repo",):
    if os.path.isdir(_p) and _p not in sys.path:
        sys.path.append(_p)


def _ensure_axon_hooks():
    """Make antenv.axon_hooks importable so bass_utils trace mode never
    crashes on the import (hook may still be None -> tracing is skipped)."""
    try:
        import antenv.axon_hooks  # noqa: F401
        return
    except Exception:
        pass
    try:
        import antenv
        mod = types.ModuleType("antenv.axon_hooks")
        mod._hook = None

        def set_axon_ntff_profile_hook(hook):
            mod._hook = hook

        def get_axon_ntff_profile_hook():
            return mod._hook

        mod.set_axon_ntff_profile_hook = set_axon_ntff_profile_hook
        mod.get_axon_ntff_profile_hook = get_axon_ntff_profile_hook
        sys.modules["antenv.axon_hooks"] = mod
        antenv.axon_hooks = mod
    except Exception:
        pass


_ensure_axon_hooks()

import numpy as np  # noqa: E402

NCH = 128          # channels
W = H = 64         # spatial
S = W * H          # 4096 spatial positions
B = 2              # batch
G = 4              # cores per sample
NCORES = 8
LS = S // G        # kernels per core (1024)
LT = LS // 128     # l-tiles per core (8)
ROWS = 8           # patch-center rows per chunk
CS = ROWS * H      # spatial chunk (512)
NCHUNK = W // ROWS # 8 chunks
EPS = 1e-7

_CACHE = {}
LAST_EXEC_TIME_NS = None


def _build():
    from concourse import bacc, tile, mybir
    from concourse.masks import make_identity

    F32 = mybir.dt.float32
    BF = mybir.dt.bfloat16
    Alu = mybir.AluOpType
    Act = mybir.ActivationFunctionType
    AxX = mybir.AxisListType.X

    nc = bacc.Bacc("TRN2", target_bir_lowering=False, debug=False,
                   num_devices=NCORES)

    fg_ext = nc.dram_tensor("fg", [NCH, S], F32, kind="ExternalInput")
    fgband_ext = nc.dram_tensor("fgband", [NCH, 18 * H], F32,
                                kind="ExternalInput")
    mask_ext = nc.dram_tensor("mask", [1, S], F32, kind="ExternalInput")
    mband_ext = nc.dram_tensor("maskband", [1, 18 * H], F32,
                               kind="ExternalInput")
    out_ext = nc.dram_tensor("out", [NCH // G, S], F32, kind="ExternalOutput")

    groups = [[0, 1, 2, 3], [4, 5, 6, 7]]

    with tile.TileContext(nc) as tc:
        with tc.tile_pool(name="const", bufs=1) as cpool, \
             tc.tile_pool(name="pers", bufs=1) as pers, \
             tc.tile_pool(name="big", bufs=1) as big, \
             tc.tile_pool(name="prep", bufs=1) as prep, \
             tc.tile_pool(name="chunk", bufs=2) as wk, \
             tc.tile_pool(name="blend", bufs=2) as bl, \
             tc.tile_pool(name="psA", bufs=3, space="PSUM") as psA, \
             tc.tile_pool(name="psT", bufs=2, space="PSUM") as psT, \
             tc.tile_pool(name="psS", bufs=2, space="PSUM") as psS, \
             tc.tile_pool(name="dram", bufs=2, space="DRAM") as dram, \
             tc.tile_pool(name="dramP", bufs=1, space="DRAM") as dramP:

            ident_f = cpool.tile([128, 128], F32, tag="idf")
            make_identity(nc, ident_f[:])
            ident_b = cpool.tile([128, 128], BF, tag="idb")
            make_identity(nc, ident_b[:])
            ones_c = cpool.tile([128, 1], F32, tag="ones")
            nc.gpsimd.memset(ones_c[:], 1.0)

            # ---------------- persistent tensors ----------------
            boxbf = pers.tile([NCH, 66, 66], BF, tag="boxbf")
            kernT = pers.tile([NCH, 9, LS], BF, tag="kernT")
            kern_lc = pers.tile([128, 9, LT, NCH], BF, tag="kernlc")
            rnorm = pers.tile([128, LT], F32, tag="rnorm")

            # big66 slot: featp2 first, later vtmp, later canvas66
            featp2 = big.tile([NCH, 68, 68], F32, tag="big66")

            # ---------------- prep: boxfeat ----------------
            nc.gpsimd.memset(featp2[:], 0.0)
            nc.sync.dma_start(
                featp2[:, 2:2 + W, 2:2 + H],
                fg_ext[:].rearrange("c (y x) -> c y x", y=W))
            # horizontal 3-tap
            tmpH = prep.tile([NCH, 68, 66], BF, tag="tmpH")
            nc.vector.tensor_add(tmpH[:], featp2[:, :, 0:66], featp2[:, :, 1:67])
            nc.vector.tensor_add(tmpH[:], tmpH[:], featp2[:, :, 2:68])
            # vertical 3-tap
            vtmp = big.tile([NCH, 66, 66], BF, tag="big66")
            nc.vector.tensor_add(vtmp[:], tmpH[:, 0:66, :], tmpH[:, 1:67, :])
            nc.vector.tensor_add(boxbf[:], vtmp[:], tmpH[:, 2:68, :])

            # ---------------- prep: kernels ----------------
            fgband_sb = prep.tile([NCH, 18, H], F32, tag="fgband")
            nc.sync.dma_start(
                fgband_sb[:],
                fgband_ext[:].rearrange("c (r x) -> c r x", r=18))
            mband_row = prep.tile([1, 18 * H], F32, tag="mbandrow")
            nc.sync.dma_start(mband_row[:], mband_ext[:])
            mband_bc = prep.tile([NCH, 18 * H], F32, tag="mbandbc")
            nc.gpsimd.partition_broadcast(mband_bc[:], mband_row[:])
            bgbandp = prep.tile([NCH, 18, 66], F32, tag="bgbandp")
            nc.gpsimd.memset(bgbandp[:], 0.0)
            nc.vector.tensor_mul(
                bgbandp[:, :, 1:65], fgband_sb[:],
                mband_bc[:].rearrange("c (r x) -> c r x", r=18))
            for d in range(9):
                dy, dx = d // 3, d % 3
                nc.vector.tensor_scalar_add(
                    kernT[:, d, :],
                    bgbandp[:, dy:dy + 16, dx:dx + 64], EPS)

            # kernel norms: sumsq over (c, dydx) via ones-matmul, per l
            ps_s0 = psS.tile([1, 512], F32, tag="psS")
            ps_s1 = psS.tile([1, 512], F32, tag="psS")
            for d in range(9):
                ksq0 = wk.tile([NCH, 512], F32, tag="sc0")
                ksq1 = wk.tile([NCH, 512], F32, tag="sc1")
                nc.scalar.activation(ksq0[:], kernT[:, d, 0:512], Act.Square)
                nc.scalar.activation(ksq1[:], kernT[:, d, 512:1024],
                                     Act.Square)
                nc.tensor.matmul(ps_s0[:], ones_c[:], ksq0[:],
                                 start=(d == 0), stop=(d == 8))
                nc.tensor.matmul(ps_s1[:], ones_c[:], ksq1[:],
                                 start=(d == 0), stop=(d == 8))
            rnorm_row = prep.tile([1, LS], F32, tag="rnormrow")
            norm_row = prep.tile([1, LS], F32, tag="normrow")
            nc.scalar.activation(norm_row[:, 0:512], ps_s0[:], Act.Sqrt)
            nc.scalar.activation(norm_row[:, 512:1024], ps_s1[:], Act.Sqrt)
            nc.vector.reciprocal(rnorm_row[:], norm_row[:])
            # scatter row -> [128 partitions, LT] column layout (l = t*128 + p)
            # via a DRAM bounce (SBUF->SBUF cannot rebalance partition dims)
            rn_dram = dram.tile([LS], F32, tag="rnd")
            nc.sync.dma_start(rn_dram[:], rnorm_row[:])
            nc.sync.dma_start(
                rnorm[:],
                rn_dram[:].rearrange("(t p) -> p t", t=LT, p=128))

            # kern_lc: per (d, t) PE-transpose kernT tile [c,128l] -> [128l, c]
            for d in range(9):
                for t in range(LT):
                    pt = psT.tile([128, 128], BF, tag="psT")
                    nc.tensor.transpose(
                        pt[:], kernT[:, d, t * 128:(t + 1) * 128], ident_b[:])
                    nc.vector.tensor_copy(kern_lc[:, d, t, :], pt[:])

            # canvas (shares the big66 slot; zeroed before folds)
            canvas = big.tile([NCH, 66, 66], F32, tag="big66")
            nc.gpsimd.memset(canvas[:], 0.0)

            canvas_in = dramP.tile([NCH, S], F32, tag="cin")
            rs_out = dramP.tile([NCH // G, S], F32, tag="rsout")

            # ---------------- main chunk loop ----------------
            for k in range(NCHUNK):
                r0 = k * ROWS
                # GEMM1: scores for this chunk
                scs = []
                for t in range(LT):
                    ps = psA.tile([128, CS], F32, tag="psA")
                    for d in range(9):
                        dy, dx = d // 3, d % 3
                        nc.tensor.matmul(
                            ps[:],
                            kernT[:, d, t * 128:(t + 1) * 128],
                            boxbf[:, r0 + dy:r0 + dy + ROWS, dx:dx + 64],
                            start=(d == 0), stop=(d == 8))
                    sc = wk.tile([128, CS], F32, tag=f"sc{t}")
                    nc.vector.tensor_scalar_mul(sc[:], ps[:], rnorm[:, t:t + 1])
                    scs.append(sc)

                # local max over l (partition axis) via transpose
                mtmp = wk.tile([128, CS], F32, tag="mtmp")
                nc.vector.tensor_max(mtmp[:], scs[0][:], scs[1][:])
                for t in range(2, LT):
                    nc.vector.tensor_max(mtmp[:], mtmp[:], scs[t][:])
                mloc = wk.tile([128, CS // 128], F32, tag="mloc")
                for j in range(CS // 128):
                    pt = psT.tile([128, 128], F32, tag="psT")
                    nc.tensor.transpose(
                        pt[:], mtmp[:, j * 128:(j + 1) * 128], ident_f[:])
                    nc.vector.tensor_reduce(mloc[:, j:j + 1], pt[:], AxX,
                                            Alu.max)
                st_max_i = dram.tile([CS], F32, tag="stmi")
                st_max_o = dram.tile([CS], F32, tag="stmo")
                nc.gpsimd.dma_start(
                    st_max_i[:].rearrange("(t p) -> p t", t=CS // 128, p=128),
                    mloc[:])
                nc.gpsimd.collective_compute(
                    "AllReduce", Alu.max, replica_groups=groups,
                    ins=[st_max_i.opt()], outs=[st_max_o.opt()])
                gmax_row = wk.tile([1, CS], F32, tag="gmaxrow")
                nc.gpsimd.dma_start(gmax_row[:], st_max_o[:])
                gmax_bc = wk.tile([128, CS], F32, tag="gmaxbc")
                nc.gpsimd.partition_broadcast(gmax_bc[:], gmax_row[:])

                # exp(score - gmax) in place
                for t in range(LT):
                    nc.vector.tensor_sub(scs[t][:], scs[t][:], gmax_bc[:])
                    nc.scalar.activation(scs[t][:], scs[t][:], Act.Exp)

                # local sum over l via transpose
                stmp = wk.tile([128, CS], F32, tag="mtmp")
                nc.vector.tensor_add(stmp[:], scs[0][:], scs[1][:])
                for t in range(2, LT):
                    nc.vector.tensor_add(stmp[:], stmp[:], scs[t][:])
                sloc = wk.tile([128, CS // 128], F32, tag="sloc")
                for j in range(CS // 128):
                    pt = psT.tile([128, 128], F32, tag="psT")
                    nc.tensor.transpose(
                        pt[:], stmp[:, j * 128:(j + 1) * 128], ident_f[:])
                    nc.vector.tensor_reduce(sloc[:, j:j + 1], pt[:], AxX,
                                            Alu.add)
                st_sum_i = dram.tile([CS], F32, tag="stsi")
                st_sum_o = dram.tile([CS], F32, tag="stso")
                nc.gpsimd.dma_start(
                    st_sum_i[:].rearrange("(t p) -> p t", t=CS // 128, p=128),
                    sloc[:])
                nc.gpsimd.collective_compute(
                    "AllReduce", Alu.add, replica_groups=groups,
                    ins=[st_sum_i.opt()], outs=[st_sum_o.opt()])
                gsum_row = wk.tile([1, CS], F32, tag="gmaxrow")
                nc.gpsimd.dma_start(gsum_row[:], st_sum_o[:])
                rg_row = wk.tile([1, CS], F32, tag="rgrow")
                nc.vector.reciprocal(rg_row[:], gsum_row[:])
                rg_bc = wk.tile([128, CS], F32, tag="gmaxbc")
                nc.gpsimd.partition_broadcast(rg_bc[:], rg_row[:])

                # attn (bf16) = exps * rnorm[l] * (1/gsum[s])
                ats = []
                for t in range(LT):
                    at = wk.tile([128, CS], BF, tag=f"at{t}")
                    nc.vector.scalar_tensor_tensor(
                        at[:], scs[t][:], rnorm[:, t:t + 1], rg_bc[:],
                        op0=Alu.mult, op1=Alu.mult)
                    ats.append(at)

                # GEMM2 + fold into canvas
                for d in range(9):
                    dy, dx = d // 3, d % 3
                    ps2 = psA.tile([128, CS], F32, tag="psA")
                    for t in range(LT):
                        nc.tensor.matmul(
                            ps2[:], kern_lc[:, d, t, :], ats[t][:],
                            start=(t == 0), stop=(t == LT - 1))
                    csl = canvas[:, r0 + dy:r0 + dy + ROWS, dx:dx + 64]
                    nc.vector.tensor_add(
                        csl, csl,
                        ps2[:].rearrange("p (r x) -> p r x", r=ROWS))

            # ---------------- blend + ReduceScatter ----------------
            for k in range(NCHUNK):
                r0 = k * ROWS
                cint = canvas[:, 1 + r0:1 + r0 + ROWS, 1:65]
                mrow = bl.tile([1, CS], F32, tag="mrow")
                nc.sync.dma_start(mrow[:], mask_ext[:, k * CS:(k + 1) * CS])
                mbc = bl.tile([128, CS], F32, tag="mbc")
                nc.gpsimd.partition_broadcast(mbc[:], mrow[:])
                fgc = bl.tile([NCH, CS], F32, tag="fgc")
                nc.sync.dma_start(fgc[:], fg_ext[:, k * CS:(k + 1) * CS])
                mc = bl.tile([128, CS], F32, tag="mc")
                mc3 = mc[:].rearrange("p (r x) -> p r x", r=ROWS)
                nc.vector.tensor_mul(
                    mc3, cint, mbc[:].rearrange("p (r x) -> p r x", r=ROWS))
                nc.vector.tensor_sub(mc3, cint, mc3)
                mf = bl.tile([128, CS], F32, tag="mf")
                nc.vector.scalar_tensor_tensor(
                    mf[:], fgc[:], 1.0 / G, mbc[:], op0=Alu.mult, op1=Alu.mult)
                outb = bl.tile([128, CS], F32, tag="mc")
                nc.vector.scalar_tensor_tensor(
                    outb[:], mc[:], 1.0 / 9.0, mf[:], op0=Alu.mult,
                    op1=Alu.add)
                nc.sync.dma_start(canvas_in[:, k * CS:(k + 1) * CS], outb[:])

            nc.gpsimd.collective_compute(
                "ReduceScatter", mybir.AluOpType.add, replica_groups=groups,
                ins=[canvas_in.opt()], outs=[rs_out.opt()])
            nc.sync.dma_start(out_ext[:], rs_out[:])

    nc.compile()
    return nc


def _shard_inputs(fg, mk):
    """fg [2,128,64,64] f32, mk [2,1,64,64] f32 -> per-core input maps."""
    in_maps = []
    for core in range(NCORES):
        b, r = core // G, core % G
        y0 = r * (W // G)
        feat = np.ascontiguousarray(fg[b].reshape(NCH, S), np.float32)
        mask = np.ascontiguousarray(mk[b].reshape(1, S), np.float32)
        band = np.zeros((NCH, 18, H), np.float32)
        mband = np.zeros((1, 18, H), np.float32)
        lo = y0 - 1
        src_lo = max(0, lo)
        src_hi = min(W, y0 + 17)
        band[:, src_lo - lo:src_hi - lo] = fg[b][:, src_lo:src_hi]
        mband[:, src_lo - lo:src_hi - lo] = mk[b][:, src_lo:src_hi]
        in_maps.append({
            "fg": feat,
            "fgband": np.ascontiguousarray(band.reshape(NCH, 18 * H)),
            "mask": mask,
            "maskband": np.ascontiguousarray(mband.reshape(1, 18 * H)),
        })
    return in_maps


def kernel(foreground, masks):
    global LAST_EXEC_TIME_NS
    from concourse.bass_utils import run_bass_kernel_spmd

    fg = np.asarray(foreground, np.float32)
    mk = np.asarray(masks, np.float32)
    assert fg.shape == (B, NCH, W, H) and mk.shape == (B, 1, W, H)

    nc = _CACHE.get("nc")
    if nc is None:
        nc = _build()
        _CACHE["nc"] = nc

    in_maps = _shard_inputs(fg, mk)
    trace = bool(os.environ.get("BASS_KERNEL_TRACE"))
    res = run_bass_kernel_spmd(nc, in_maps, core_ids=list(range(NCORES)),
                               trace=trace)
    LAST_EXEC_TIME_NS = res.exec_time_ns
    if res.exec_time_ns is not None:
        print(f"HW exec time: {res.exec_time_ns} ns")

    out = np.empty((B, NCH, W, H), np.float32)
    for core in range(NCORES):
        b, r = core // G, core % G
        out[b, 32 * r:32 * (r + 1)] = (
            res.results[core]["out"].reshape(32, W, H))
    return out
